# revision 5
# baseline (speedup 1.0000x reference)
"""3-layer GCN (DGI) forward on 8 Trainium2 NeuronCores.

Strategy: the normalized adjacency S = D^-1/2 (A+I) D^-1/2 is applied as a
*dense* block matmul on the tensor engine.  We keep the adjacency **binary**
(with edge multiplicities) so its bf16 representation is exact, and fold the
two degree scalings into per-partition activation scales:

    h_out = prelu( dinv_t * ( A @ (dinv_s * (h_in @ W)) ) + b )

Sharding: nodes (rows of the output) are sharded 8 ways, 1280 per core (N is
padded 10000 -> 10240 with isolated phantom nodes).  Each core streams its
[1280 x 10240] bf16 slice of A from HBM and computes S @ Z for its rows; the
transformed features Z (both seq1 and seq2 fused, 256 feats) are exchanged
with an AllGather each layer.  seq1/seq2 share A and the weights, so both
propagate in the same matmuls.

Layouts (SBUF is [partition, free]):
  h_all   [128 t_part, 10 tb * 256 f]   per-core activations, bf16
  hT      [128 f_part, 1280 t]          transposed (per seq) for the XW matmul
  Zf      [128 s_part, 80 sb * 256 f]   all-gathered features, bf16
  At      [128 s_part, 80 sb * 128 t]   adjacency tile for one target block
"""

import numpy as np
import ml_dtypes

import concourse.bass as bass
import concourse.bacc as bacc
import concourse.mybir as mybir
import concourse.tile as tile
from concourse import bass_utils

BF16 = ml_dtypes.bfloat16

N = 10000          # real nodes
C = 8              # cores
T = 1280           # nodes per core (padded)
NP = C * T         # padded node count 10240
NBT = T // 128     # target blocks per core (10)
NBS = NP // 128    # source blocks (80)
D = 128            # feature dim per sequence
F = 2 * D          # fused feature dim (seq1 | seq2)

_prog_cache = {}


def _build_program(a_prelu: float, b_bilin: float, has_bias: bool):
    f32 = mybir.dt.float32
    bf16 = mybir.dt.bfloat16
    AF = mybir.ActivationFunctionType

    nc = bacc.Bacc("TRN2", target_bir_lowering=False, debug=False, num_devices=C)

    At_d = nc.dram_tensor("At", [NBT, 128, NBS * 128], bf16, kind="ExternalInput")
    XT1_d = nc.dram_tensor("XT1", [128, T], bf16, kind="ExternalInput")
    XT2_d = nc.dram_tensor("XT2", [128, T], bf16, kind="ExternalInput")
    dinv_d = nc.dram_tensor("dinv", [128, NBT], f32, kind="ExternalInput")
    mask_d = nc.dram_tensor("mask", [128, NBT], bf16, kind="ExternalInput")
    W_d = nc.dram_tensor("W", [3, 128, 128], bf16, kind="ExternalInput")
    WbT_d = nc.dram_tensor("WbT", [128, 128], f32, kind="ExternalInput")
    ident_d = nc.dram_tensor("ident", [128, 128], bf16, kind="ExternalInput")
    if has_bias:
        ones_d = nc.dram_tensor("ones1", [1, 128], f32, kind="ExternalInput")
        b_d = nc.dram_tensor("b", [3, 1, F], f32, kind="ExternalInput")
    out_d = nc.dram_tensor("out", [128, 2 * NBT], f32, kind="ExternalOutput")

    ag_in = [nc.dram_tensor(f"agin{l}", [128, NBT * F], bf16) for l in range(3)]
    ag_out = [
        nc.dram_tensor(f"agout{l}", [C * 128, NBT * F], bf16, addr_space="Shared")
        for l in range(3)
    ]
    ar_in = nc.dram_tensor("arin", [128, 1], f32)
    ar_out = nc.dram_tensor("arout", [128, 1], f32, addr_space="Shared")
    rg = [list(range(C))]

    with tile.TileContext(nc) as tc:
        with (
            tc.tile_pool(name="sb", bufs=2) as sb,
            tc.tile_pool(name="stat", bufs=1) as stat,
            tc.tile_pool(name="psS", bufs=2, space="PSUM") as psS,
            tc.tile_pool(name="psU", bufs=4, space="PSUM") as psU,
            tc.tile_pool(name="psT", bufs=2, space="PSUM") as psT,
        ):
            # ---- static tiles ----
            xt1 = stat.tile([128, T], bf16, tag="xt1")
            xt2 = stat.tile([128, T], bf16, tag="xt2")
            nc.sync.dma_start(xt1[:], XT1_d[:, :])
            nc.sync.dma_start(xt2[:], XT2_d[:, :])
            dinv_sb = stat.tile([128, NBT], f32, tag="dinv")
            nc.sync.dma_start(dinv_sb[:], dinv_d[:, :])
            mask_sb = stat.tile([128, NBT], bf16, tag="mask")
            nc.sync.dma_start(mask_sb[:], mask_d[:, :])
            W_sb = stat.tile([128, 3 * 128], bf16, tag="W")
            for l in range(3):
                nc.sync.dma_start(W_sb[:, l * 128:(l + 1) * 128], W_d[l, :, :])
            WbT_sb = stat.tile([128, 128], f32, tag="WbT")
            nc.sync.dma_start(WbT_sb[:], WbT_d[:, :])
            ident_sb = stat.tile([128, 128], bf16, tag="ident")
            nc.sync.dma_start(ident_sb[:], ident_d[:, :])

            bias_sb = None
            if has_bias:
                ones_sb = stat.tile([1, 128], f32, tag="ones1")
                nc.sync.dma_start(ones_sb[:], ones_d[:, :])
                b_sb = stat.tile([1, 3 * F], f32, tag="bvec")
                for l in range(3):
                    nc.sync.dma_start(b_sb[:, l * F:(l + 1) * F], b_d[l, :, :])
                bias_sb = stat.tile([128, 3 * F], f32, tag="btile")
                for l in range(3):
                    b_ps = psU.tile([128, F], f32, tag="u")
                    nc.tensor.matmul(
                        b_ps[:], ones_sb[:], b_sb[:, l * F:(l + 1) * F],
                        start=True, stop=True,
                    )
                    nc.vector.tensor_copy(bias_sb[:, l * F:(l + 1) * F], b_ps[:])

            hT = [xt1, xt2]
            h_all = None
            for l in range(3):
                # ---- XW + degree scale -> Z, staged for AllGather ----
                z_sb = sb.tile([128, NBT * F], bf16, tag="z")
                for tb in range(NBT):
                    for s in range(2):
                        u_ps = psU.tile([128, 128], f32, tag="u")
                        nc.tensor.matmul(
                            u_ps[:],
                            hT[s][:, tb * 128:(tb + 1) * 128],
                            W_sb[:, l * 128:(l + 1) * 128],
                            start=True, stop=True,
                        )
                        nc.scalar.activation(
                            z_sb[:, tb * F + s * 128: tb * F + s * 128 + 128],
                            u_ps[:], AF.Copy, scale=dinv_sb[:, tb:tb + 1],
                        )
                nc.sync.dma_start(ag_in[l][:, :], z_sb[:])
                nc.gpsimd.collective_compute(
                    "AllGather", mybir.AluOpType.bypass, replica_groups=rg,
                    ins=[ag_in[l].ap().opt()], outs=[ag_out[l].ap().opt()],
                )
                zf = sb.tile([128, NBS * F], bf16, tag="zf")
                W_AG = NBT * F
                for r in range(C):
                    nc.sync.dma_start(
                        zf[:, r * W_AG:(r + 1) * W_AG],
                        ag_out[l][r * 128:(r + 1) * 128, :],
                    )

                # ---- dense S @ Z over target blocks ----
                h_all = sb.tile([128, NBT * F], bf16, tag="h")
                hT_new = [
                    sb.tile([128, T], bf16, tag=f"hT{s}", name=f"hT{s}")
                    for s in range(2)
                ]
                for tb in range(NBT):
                    at_sb = sb.tile([128, NBS * 128], bf16, tag="at")
                    nc.sync.dma_start(at_sb[:], At_d[tb, :, :])
                    s_ps = psS.tile([128, F], f32, tag="s")
                    for sbk in range(NBS):
                        nc.tensor.matmul(
                            s_ps[:],
                            at_sb[:, sbk * 128:(sbk + 1) * 128],
                            zf[:, sbk * F:(sbk + 1) * F],
                            start=(sbk == 0), stop=(sbk == NBS - 1),
                        )
                    hslc = h_all[:, tb * F:(tb + 1) * F]
                    if has_bias:
                        p1 = sb.tile([128, F], f32, tag="p1")
                        nc.scalar.activation(
                            p1[:], s_ps[:], AF.Copy, scale=dinv_sb[:, tb:tb + 1]
                        )
                        p2 = sb.tile([128, F], f32, tag="p2")
                        nc.vector.tensor_add(
                            p2[:], p1[:], bias_sb[:, l * F:(l + 1) * F]
                        )
                        nc.scalar.activation(hslc, p2[:], AF.Prelu, alpha=a_prelu)
                    else:
                        nc.scalar.activation(
                            hslc, s_ps[:], AF.Prelu,
                            scale=dinv_sb[:, tb:tb + 1], alpha=a_prelu,
                        )
                    # transpose for next layer's XW (and final scores)
                    for s in range(2):
                        tr_ps = psT.tile([128, 128], bf16, tag="tr")
                        nc.tensor.transpose(
                            tr_ps[:],
                            h_all[:, tb * F + s * 128: tb * F + s * 128 + 128],
                            ident_sb[:],
                        )
                        nc.vector.tensor_copy(
                            hT_new[s][:, tb * 128:(tb + 1) * 128], tr_ps[:]
                        )
                hT = hT_new

            # ---- readout: c = sigmoid(mean(h1)); wc = W_bilin @ c ----
            cs_ps = psU.tile([128, 1], f32, tag="u")
            for tb in range(NBT):
                nc.tensor.matmul(
                    cs_ps[:],
                    h_all[:, tb * F: tb * F + 128],
                    mask_sb[:, tb:tb + 1],
                    start=(tb == 0), stop=(tb == NBT - 1),
                )
            cs_sb = sb.tile([128, 1], f32, tag="cs")
            nc.vector.tensor_copy(cs_sb[:], cs_ps[:])
            nc.sync.dma_start(ar_in[:, :], cs_sb[:])
            nc.gpsimd.collective_compute(
                "AllReduce", mybir.AluOpType.add, replica_groups=rg,
                ins=[ar_in.ap().opt()], outs=[ar_out.ap().opt()],
            )
            csum = sb.tile([128, 1], f32, tag="csum")
            nc.sync.dma_start(csum[:], ar_out[:, :])
            c_sb = sb.tile([128, 1], f32, tag="c")
            nc.scalar.activation(c_sb[:], csum[:], AF.Sigmoid, scale=1.0 / N)
            wc_ps = psU.tile([128, 1], f32, tag="u")
            nc.tensor.matmul(wc_ps[:], WbT_sb[:], c_sb[:], start=True, stop=True)
            wc_bf = sb.tile([128, 1], bf16, tag="wc")
            nc.vector.tensor_copy(wc_bf[:], wc_ps[:])

            # ---- scores sc = h3 @ wc + b_bilin ----
            out_sb = sb.tile([128, 2 * NBT], f32, tag="out")
            for s in range(2):
                for tb in range(NBT):
                    sc_ps = psU.tile([128, 1], f32, tag="u")
                    nc.tensor.matmul(
                        sc_ps[:], hT[s][:, tb * 128:(tb + 1) * 128], wc_bf[:],
                        start=True, stop=True,
                    )
                    nc.scalar.activation(
                        out_sb[:, s * NBT + tb: s * NBT + tb + 1],
                        sc_ps[:], AF.Identity, bias=b_bilin,
                    )
            nc.sync.dma_start(out_d[:, :], out_sb[:])

    nc.compile()
    return nc


def _prepare_inputs(seq1, seq2, edge_index, W1, b1, W2, b2, W3, b3,
                    a_prelu, W_bilin, b_bilin):
    row = np.asarray(edge_index[0], dtype=np.int64)
    col = np.asarray(edge_index[1], dtype=np.int64)

    deg = np.bincount(col, minlength=N).astype(np.float32) + 1.0
    dinv = (1.0 / np.sqrt(deg)).astype(np.float32)
    dinv_pad = np.zeros(NP, np.float32)
    dinv_pad[:N] = dinv
    maskv = np.zeros(NP, np.float32)
    maskv[:N] = 1.0

    # adjacency with multiplicities + self loops; A[t, s]
    A = np.zeros((NP, NP), dtype=np.float32)
    np.add.at(A, (col, row), 1.0)
    idx = np.arange(N)
    A[idx, idx] += 1.0
    Abf = A.astype(BF16)

    X1 = np.zeros((NP, D), np.float32)
    X1[:N] = np.asarray(seq1, np.float32)
    X2 = np.zeros((NP, D), np.float32)
    X2[:N] = np.asarray(seq2, np.float32)

    Wcat = np.stack([
        np.asarray(W1, np.float32),
        np.asarray(W2, np.float32),
        np.asarray(W3, np.float32),
    ]).astype(BF16)
    bcat = np.stack([
        np.concatenate([np.asarray(b1, np.float32)] * 2),
        np.concatenate([np.asarray(b2, np.float32)] * 2),
        np.concatenate([np.asarray(b3, np.float32)] * 2),
    ]).astype(np.float32).reshape(3, 1, F)
    has_bias = bool(np.any(bcat != 0.0))

    WbT = np.ascontiguousarray(np.asarray(W_bilin, np.float32).T)
    ident = np.eye(128, dtype=np.float32).astype(BF16)
    ones1 = np.ones((1, 128), np.float32)

    in_maps = []
    for c in range(C):
        t0 = c * T
        At_c = np.ascontiguousarray(
            Abf[t0:t0 + T, :]
            .reshape(NBT, 128, NBS, 128)
            .transpose(0, 3, 2, 1)
        ).reshape(NBT, 128, NBS * 128)
        m = {
            "At": At_c,
            "XT1": np.ascontiguousarray(X1[t0:t0 + T].T).astype(BF16),
            "XT2": np.ascontiguousarray(X2[t0:t0 + T].T).astype(BF16),
            "dinv": np.ascontiguousarray(dinv_pad[t0:t0 + T].reshape(NBT, 128).T),
            "mask": np.ascontiguousarray(
                maskv[t0:t0 + T].reshape(NBT, 128).T).astype(BF16),
            "W": Wcat,
            "WbT": WbT,
            "ident": ident,
        }
        if has_bias:
            m["b"] = bcat
            m["ones1"] = ones1
        in_maps.append(m)
    return in_maps, has_bias, float(a_prelu), float(b_bilin)


def _run(in_maps, has_bias, a_prelu, b_bilin, **run_kwargs):
    key = (has_bias, a_prelu, b_bilin)
    if key not in _prog_cache:
        _prog_cache[key] = _build_program(a_prelu, b_bilin, has_bias)
    nc = _prog_cache[key]
    res = bass_utils.run_bass_kernel_spmd(
        nc, in_maps, core_ids=list(range(C)), **run_kwargs
    )
    parts = []
    for c in range(C):
        o = np.asarray(res.results[c]["out"], np.float32)     # [128, 2*NBT]
        parts.append(o.reshape(128, 2, NBT).transpose(1, 2, 0).reshape(2, T))
    sc = np.concatenate(parts, axis=1)                        # [2, NP]
    out = np.concatenate([sc[0, :N], sc[1, :N]]).astype(np.float32)
    return out, res


def kernel(**inputs):
    in_maps, has_bias, a_prelu, b_bilin = _prepare_inputs(**inputs)
    out, _ = _run(in_maps, has_bias, a_prelu, b_bilin)
    return out
